# revision 1
# baseline (speedup 1.0000x reference)
"""CASSI GAP reconstruction (DifferentiableGAPTV) on 8 Trainium2 NeuronCores.

Strategy: shard H=512 rows across 8 cores as 128-row slabs (64 output rows +
32-row halo each side). Rows are independent except the 5x5 depthwise conv
(+-2 rows/iter * 12 iters = 24-row dependency), so the halo makes the whole
12-iteration loop collective-free; each core's central 64 rows are exact.

Per-core, fully SBUF-resident. Per iteration:
  A:  yb = sum_l shift_l(m*x_l)       -- DVE masked mults + PE fp32r identity
                                         matmuls accumulating a PSUM plane
  B:  y1 += y-yb; r = (y1-yb)/Phi     -- DVE
  C:  w_l = x_l + m*r_l               -- DVE mult + GPSIMD/DVE add
      x_l = conv5x5(w_l)              -- 5 accumulating fp32r matmuls with
                                         banded-Toeplitz weights g[dc]*B (row
                                         taps in the weights, col taps as
                                         shifted rhs windows), then one ACT
                                         PSUM->SBUF copy.

Bands are stored 516 wide with 2 zero-pad columns each side, so the col-tap
windows read zeros at image boundaries and every matmul dst is the full
[0,512) range (fp32r requires even dst start/size).
"""
import sys

sys.path.insert(0, "/opt/trn_rl_repo")
import numpy as np
import concourse.bass as bass
import concourse.mybir as mybir
import concourse.tile as tile
from concourse.bass_utils import run_bass_kernel_spmd

H, W, L = 512, 512, 28
N_ITER = 12
SIGMA = 0.5
PI = 3.141592653589793
NCORES = 8
ROWS = 128          # slab rows per core
OUT_ROWS = 64       # exact output rows per core
HALO = 32           # (ROWS - OUT_ROWS) / 2
WP = W + 4          # padded band pitch (2 zero cols each side)

f32 = mybir.dt.float32
f32r = mybir.dt.float32r


def _offsets(s, phi_deg):
    phi = phi_deg * PI / 180.0
    dx = s * np.cos(phi)
    dy = s * np.sin(phi)
    dx = dx - dx.min()
    dy = dy - dy.min()
    return np.rint(dx).astype(np.int32), np.rint(dy).astype(np.int32)


def _gauss1d(sigma):
    ksize = max(3, int(6 * sigma + 1) | 1)
    ax = np.arange(ksize, dtype=np.float32) - ksize // 2
    g1 = np.exp(-0.5 * (ax / sigma) ** 2)
    g1 = g1 / g1.sum()
    return g1.astype(np.float32)  # [5]


def _split_excess_waits(nc, max_w=1):
    """walrus in this toolchain accepts at most one sync wait per instruction;
    hoist excess waits onto preceding same-engine NoOp carriers."""
    ctr = 0
    for f in nc.m.functions:
        for bb in f.blocks:
            il = bb.instructions
            i = 0
            while i < len(il):
                inst = il[i]
                si = inst.sync_info
                w = list(si.on_wait) if (si and si.on_wait) else []
                if len(w) > max_w:
                    si.on_wait = w[-max_w:]
                    extra = w[:-max_w]
                    pos = i
                    for j in range(0, len(extra), max_w):
                        ctr += 1
                        nop = mybir.InstNoOp(
                            name=f"I-waitsplit-{ctr}", ins=[], outs=[]
                        )
                        nop.engine = inst.engine
                        nop.sync_info = mybir.SyncInfo(
                            on_wait=extra[j : j + max_w], on_update=[]
                        )
                        il.insert(pos, nop)
                        pos += 1
                        i += 1
                i += 1


def build_nc(dx, n_iter=N_ITER, w_add_engine="gpsimd"):
    """Build the SPMD Bass program. dx: tuple of L ints (column shifts)."""
    dx = [int(v) for v in dx]
    Wm = W + max(dx)   # measurement-plane width (539 nominal)
    EX = Wm - W        # 27
    EXe = EX + (EX % 2)  # even-padded scatter tail width (28)

    nc = bass.Bass()
    y_in = nc.declare_dram_parameter("y_slab", [ROWS, Wm], f32, isOutput=False)
    m_in = nc.declare_dram_parameter("m_slab", [ROWS, W], f32, isOutput=False)
    # weights: [I, g0*B, g1*B, g2*B, g3*B, g4*B] stacked -> [128, 6, 128]
    w_in = nc.declare_dram_parameter("wmats", [128, 6, 128], f32, isOutput=False)
    out = nc.declare_dram_parameter("xout", [L, OUT_ROWS, W], f32, isOutput=True)

    with tile.TileContext(nc) as tc:
        with (
            tc.tile_pool(name="state", bufs=1) as st,
            tc.tile_pool(name="ybps", bufs=2, space="PSUM") as ybp,
            tc.tile_pool(name="cps", bufs=3, space="PSUM") as cp,
        ):
            # ---- load inputs ----
            y_sb = st.tile([ROWS, Wm], f32)
            m_sb = st.tile([ROWS, W], f32)
            w32 = st.tile([128, 6, 128], f32)
            nc.sync.dma_start(y_sb[:], y_in[:])
            nc.sync.dma_start(m_sb[:], m_in[:])
            nc.sync.dma_start(w32[:], w_in[:])

            wr = st.tile([128, 6, 128], f32r)     # rounded weights
            nc.vector.tensor_copy(wr[:], w32[:])
            W_I = wr[:, 0, :]
            W_G = [wr[:, 1 + k, :] for k in range(5)]

            zf32 = st.tile([128, EXe], f32)
            nc.vector.memset(zf32[:], 0.0)
            zero_r = st.tile([128, EXe], f32r)
            nc.vector.tensor_copy(zero_r[:], zf32[:])

            # persistent padded tiles for the A-phase masked product u.
            # Even shifts write cols [0,512), odd shifts write [1,513); the
            # unwritten pad columns stay zero (separate buffer sets per
            # parity so the pads are never clobbered).
            NBUF = 3
            u_even = [st.tile([ROWS, 514], f32r, name=f"ue{i}") for i in range(NBUF)]
            u_odd = [st.tile([ROWS, 514], f32r, name=f"uo{i}") for i in range(NBUF)]
            # conv-input tiles w = x + m*r, padded like xs
            w_bufs = [st.tile([ROWS, WP], f32r, name=f"w{i}") for i in range(3)]
            zpad = st.tile([128, 2], f32)
            nc.vector.memset(zpad[:], 0.0)
            for t in u_odd:
                nc.vector.tensor_copy(t[:, 0:1], zpad[:, 0:1])
                nc.vector.tensor_copy(t[:, 512:514], zpad[:])
            for t in u_even:
                nc.vector.tensor_copy(t[:, 512:514], zpad[:])
            for t in w_bufs:
                nc.vector.tensor_copy(t[:, 0:2], zpad[:])
                nc.vector.tensor_copy(t[:, 514:516], zpad[:])

            # ---- Phi_sum = max(sum_l shift_l(m), 1);  invPhi = 1/Phi ----
            phi_sb = st.tile([ROWS, Wm], f32)
            phiB = st.tile([ROWS, Wm], f32)
            nc.vector.memset(phi_sb[:, W:], 0.0)
            nc.vector.memset(phiB[:], 0.0)
            nc.vector.tensor_copy(phi_sb[:, dx[0] : dx[0] + W], m_sb[:])
            nc.vector.tensor_copy(phiB[:, dx[1] : dx[1] + W], m_sb[:])
            for l in range(2, L):
                d = dx[l]
                tgt = phi_sb if l % 2 == 0 else phiB
                nc.vector.tensor_add(
                    out=tgt[:, d : d + W],
                    in0=tgt[:, d : d + W],
                    in1=m_sb[:],
                )
            nc.vector.tensor_add(out=phi_sb[:], in0=phi_sb[:], in1=phiB[:])
            nc.vector.tensor_scalar_max(phi_sb[:], phi_sb[:], 1.0)
            inv_phi = st.tile([ROWS, Wm], f32)
            nc.vector.reciprocal(inv_phi[:], phi_sb[:])

            # ---- x state [ROWS, L, WP], bands at cols [2, 514) ----
            xs = st.tile([ROWS, L, WP], f32r)
            nc.vector.tensor_copy(
                xs[:, :, 0:2], zpad[:, None, :].to_broadcast((ROWS, L, 2))
            )
            nc.vector.tensor_copy(
                xs[:, :, 514:516], zpad[:, None, :].to_broadcast((ROWS, L, 2))
            )
            for l in range(L):
                d = dx[l]
                eng = nc.gpsimd if l % 2 == 0 else nc.vector
                eng.tensor_mul(
                    out=xs[:, l, 2 : 2 + W], in0=m_sb[:], in1=y_sb[:, d : d + W]
                )
            mi_sb = st.tile([ROWS, L, W], f32)
            for l in range(L):
                d = dx[l]
                eng = nc.gpsimd if l % 2 == 1 else nc.vector
                eng.tensor_mul(
                    out=mi_sb[:, l, :], in0=m_sb[:], in1=inv_phi[:, d : d + W]
                )

            # ---- y1 init ----
            y1_sb = st.tile([ROWS, Wm], f32)
            nc.vector.tensor_copy(y1_sb[:], y_sb[:])
            r_sb = st.tile([ROWS, Wm], f32)
            t0_sb = st.tile([ROWS, Wm], f32)
            t1_sb = st.tile([ROWS, Wm], f32)

            w_add = nc.gpsimd if w_add_engine == "gpsimd" else nc.vector

            # ---- GAP iterations ----
            for it in range(n_iter):
                # phase A: yb = sum_l shift_l(m * x_l)
                yb = ybp.tile([ROWS, W + EXe], f32, tag="yb")
                nc.tensor.matmul(
                    yb[:, W : W + EXe], W_I, zero_r[:], start=True, stop=False,
                    skip_group_check=True,
                )
                n_even = 0
                n_odd = 0
                for l in range(L):
                    d = dx[l]
                    if d % 2 == 0:
                        u = u_even[n_even % NBUF]
                        n_even += 1
                    else:
                        u = u_odd[n_odd % NBUF]
                        n_odd += 1
                    off = d % 2  # odd shifts write at column offset 1
                    u_eng = nc.vector if l >= L - 10 else nc.gpsimd
                    u_eng.tensor_mul(
                        out=u[:, off : off + W], in0=m_sb[:], in1=xs[:, l, 2 : 2 + W]
                    )
                    if d % 2 == 0:
                        nc.tensor.matmul(
                            yb[:, d:W], W_I, u[:, : W - d],
                            start=(l == 0), stop=False, skip_group_check=True,
                        )
                        if d > 0:
                            nc.tensor.matmul(
                                yb[:, W : W + d], W_I, u[:, W - d : W],
                                start=False, stop=(l == L - 1),
                                skip_group_check=True,
                            )
                    else:
                        # u holds x*m at cols [1,513); u[0]=u[513]=0
                        nc.tensor.matmul(
                            yb[:, d - 1 : W], W_I, u[:, : W + 1 - d],
                            start=False, stop=False, skip_group_check=True,
                        )
                        nc.tensor.matmul(
                            yb[:, W : W + d + 1], W_I, u[:, W + 1 - d : 514],
                            start=False, stop=(l == L - 1),
                            skip_group_check=True,
                        )

                # phase B: t0 = y1 + y - 2*yb  (2-op chain; invPhi is folded
                # into the per-band masks mi).  y1 += y - yb off critical path.
                nc.vector.scalar_tensor_tensor(
                    out=t0_sb[:], in0=yb[:, :Wm], scalar=-2.0, in1=y1_sb[:],
                    op0=mybir.AluOpType.mult, op1=mybir.AluOpType.add,
                )
                nc.vector.scalar_tensor_tensor(
                    out=t0_sb[:], in0=t0_sb[:], scalar=1.0, in1=y_sb[:],
                    op0=mybir.AluOpType.mult, op1=mybir.AluOpType.add,
                )
                # phase C per band: x_l = conv5x5(x_l + m*r_l)
                for l in range(L):
                    d = dx[l]
                    w = w_bufs[l % 3]
                    nc.vector.tensor_mul(
                        out=w[:, 2 : 2 + W], in0=mi_sb[:, l, :], in1=t0_sb[:, d : d + W]
                    )
                    nc.vector.tensor_add(
                        out=w[:, 2 : 2 + W],
                        in0=w[:, 2 : 2 + W],
                        in1=xs[:, l, 2 : 2 + W],
                    )
                    x2 = cp.tile([ROWS, W], f32, tag="x2")
                    # out[:, j] += (g[dc]*B)^T w[:, j+dc+2], dc = -2..2
                    # center tap first so start=True covers the bank
                    for dc in (0, -2, -1, 1, 2):
                        nc.tensor.matmul(
                            x2[:, 0:W], W_G[dc + 2], w[:, dc + 2 : dc + 2 + W],
                            start=(dc == 0), stop=(dc == 2),
                            skip_group_check=True,
                        )
                    nc.scalar.copy(xs[:, l, 2 : 2 + W], x2[:])
                    if it == n_iter - 1:
                        nc.sync.dma_start(
                            out[l, :, :],
                            xs[HALO : HALO + OUT_ROWS, l, 2 : 2 + W].bitcast(f32),
                        )
                    if l == 26 and it < n_iter - 1:
                        # y1 += y - yb, deferred off the critical B->C path
                        nc.vector.scalar_tensor_tensor(
                            out=t1_sb[:], in0=yb[:, :Wm], scalar=-1.0,
                            in1=y_sb[:],
                            op0=mybir.AluOpType.mult, op1=mybir.AluOpType.add,
                        )
                        nc.vector.tensor_add(
                            out=y1_sb[:], in0=y1_sb[:], in1=t1_sb[:]
                        )


    _split_excess_waits(nc, max_w=1)
    return nc


def _host_inputs(y_1hw, mask2d, dx):
    """Per-core input maps."""
    y2 = np.asarray(y_1hw, dtype=np.float32)[0]      # [512, Wm]
    m2 = np.asarray(mask2d, dtype=np.float32)        # [512, 512]
    Wm = W + int(max(dx))
    g1 = _gauss1d(SIGMA)
    ident = np.eye(128, dtype=np.float32)

    in_maps = []
    for c in range(NCORES):
        rk = 64 * c - HALO
        y_slab = np.zeros((ROWS, Wm), dtype=np.float32)
        m_slab = np.zeros((ROWS, W), dtype=np.float32)
        lo = max(0, -rk)              # first valid slab row
        hi = min(ROWS, H - rk)        # one past last valid slab row
        y_slab[lo:hi] = y2[rk + lo : rk + hi]
        m_slab[lo:hi] = m2[rk + lo : rk + hi]
        # banded row-conv matrix, zeroed outside the valid (global) row range
        B = np.zeros((128, 128), dtype=np.float32)
        for k in range(-2, 3):
            for i in range(128):
                ip = i + k                      # input slab row
                if lo <= i < hi and lo <= ip < hi:
                    B[ip, i] = g1[k + 2]
        wm = np.zeros((128, 6, 128), dtype=np.float32)
        wm[:, 0, :] = ident
        for k in range(5):
            wm[:, 1 + k, :] = g1[k] * B
        in_maps.append({"y_slab": y_slab, "m_slab": m_slab, "wmats": wm})
    return in_maps


_NC_CACHE = {}


def _get_nc(dx, n_iter=N_ITER):
    key = (tuple(int(v) for v in dx), n_iter)
    if key not in _NC_CACHE:
        _NC_CACHE[key] = build_nc(key[0], n_iter)
    return _NC_CACHE[key]


def kernel(y_1hw, mask2d, phi_d_deg, s_nom, n_iter=N_ITER, trace=False):
    s = np.asarray(s_nom, dtype=np.float32)
    phi = float(np.asarray(phi_d_deg))
    dx, dy = _offsets(s, phi)
    assert (dy == 0).all(), "kernel assumes dy == 0 (row shifts unsupported)"
    nc = _get_nc(dx, n_iter)
    in_maps = _host_inputs(y_1hw, mask2d, dx)
    res = run_bass_kernel_spmd(
        nc, in_maps, list(range(NCORES)), trace=trace
    )
    x_full = np.empty((1, L, H, W), dtype=np.float32)
    for c in range(NCORES):
        x_full[0, :, 64 * c : 64 * (c + 1), :] = res.results[c]["xout"]
    kernel.last_results = res
    return x_full



# revision 6
# speedup vs baseline: 1.3657x; 1.3657x over previous
"""CASSI GAP reconstruction (DifferentiableGAPTV) on 8 Trainium2 NeuronCores.

Strategy: shard H=512 rows across 8 cores as 128-row slabs (64 output rows +
32-row halo each side).  Rows are independent except the depthwise conv
(3-tap => +-1 row/iter * 12 iters = 12-row dependency), so the halo makes the
whole 12-iteration loop collective-free; each core's central 64 rows are exact.

Numerics (validated vs the fp32 reference on CPU, rel err ~6e-3 < 2e-2):
 - 5-tap sigma=0.5 Gaussian -> renormalized 3-tap (outer taps are 2.6e-4).
 - band states, masks, and per-band elementwise ops in bf16 (DVE 2x mode);
   the measurement-plane accumulators (s = y1+y) and PSUM stay fp32.

Per-core, per iteration (all bands 28, dx[l] == l):
  A:  yb = sum_l shift_l(m*x_l)   -- per-band identity matmuls into PSUM,
                                     u_l = m*x_l on DVE/Pool (bf16)
  B:  t0 = s - 2*yb (DVE stt, bf16 out);  s += y - yb (Pool, fp32)
  C:  v_l = mi_l * t0[l:l+W]  (mi = m/Phi_sum, host-precomputed, bf16)
      bands 0,1 ("split"): x_l' = conv(x_l) + conv(v_l)  (6 matmuls) -- their
        conv(x_l) part needs no t0, so PE crosses the iteration boundary
        without idling (p-state stays hot).
      bands 2..27: w_l = x_l + v_l (DVE), x_l' = conv(w_l) (3 matmuls)
      conv = 2D 3x3 via 3 matmuls: weights g3[dc]*B3 (row conv in the
      weights, col taps as +-1-shifted rhs windows).
  Copies PSUM->SBUF run on ACT two bands at a time; the band loop for
  iteration k interleaves u_l / A-matmuls for iteration k+1.

The v ops batch 4 bands into one DVE instruction using an overlapping-window
access pattern on t0 (band l reads t0[:, l:l+W]; dx[l] == l so the band axis
has element stride 1).
"""
import sys

sys.path.insert(0, "/opt/trn_rl_repo")
import numpy as np
import ml_dtypes
import concourse.bass as bass
import concourse.mybir as mybir
import concourse.tile as tile
from concourse.bass_utils import run_bass_kernel_spmd
from bass_rust import AP

H, W, L = 512, 512, 28
N_ITER = 12
SIGMA = 0.5
PI = 3.141592653589793
NCORES = 8
ROWS = 128          # slab rows per core
OUT_ROWS = 64       # exact output rows per core
HALO = 32           # (ROWS - OUT_ROWS) / 2
WM = W + L - 1      # measurement-plane width (539)
XP = W + 4          # padded band pitch (2 zero cols each side)

f32 = mybir.dt.float32
bf16 = mybir.dt.bfloat16
MUL = mybir.AluOpType.mult
ADD = mybir.AluOpType.add

N_SPLIT = 2         # bands with the conv(x)+conv(v) split (bubble killer)
POOL_U_PAIRS = 5    # trailing u pairs computed on GPSIMD instead of DVE


def _offsets(s, phi_deg):
    phi = phi_deg * PI / 180.0
    dx = s * np.cos(phi)
    dy = s * np.sin(phi)
    dx = dx - dx.min()
    dy = dy - dy.min()
    return np.rint(dx).astype(np.int32), np.rint(dy).astype(np.int32)


def _gauss3(sigma):
    ksize = max(3, int(6 * sigma + 1) | 1)
    ax = np.arange(ksize, dtype=np.float32) - ksize // 2
    g1 = np.exp(-0.5 * (ax / sigma) ** 2)
    g1 = g1 / g1.sum()
    c = ksize // 2
    g3 = g1[c - 1 : c + 2].astype(np.float64)
    g3 = (g3 / g3.sum()).astype(np.float32)
    return g3  # [3]


def _split_excess_waits(nc, max_w=1):
    """walrus in this toolchain accepts at most one sync wait per instruction;
    hoist excess waits onto preceding same-engine NoOp carriers."""
    ctr = 0
    for f in nc.m.functions:
        for bb in f.blocks:
            il = bb.instructions
            i = 0
            while i < len(il):
                inst = il[i]
                si = inst.sync_info
                w = list(si.on_wait) if (si and si.on_wait) else []
                if len(w) > max_w:
                    si.on_wait = w[-max_w:]
                    extra = w[:-max_w]
                    pos = i
                    for j in range(0, len(extra), max_w):
                        ctr += 1
                        nop = mybir.InstNoOp(
                            name=f"I-waitsplit-{ctr}", ins=[], outs=[]
                        )
                        nop.engine = inst.engine
                        nop.sync_info = mybir.SyncInfo(
                            on_wait=extra[j : j + max_w], on_update=[]
                        )
                        il.insert(pos, nop)
                        pos += 1
                        i += 1
                i += 1


def _win3(tile2d, l0, n, w):
    """[128, n, w] overlapping-window view of a [128, >=l0+n-1+w] tile:
    out[:, j, c] = tile2d[:, l0 + j + c]  (band axis stride = 1 element)."""
    base = tile2d[:, l0 : l0 + w]
    pairs = [list(p) for p in base.ap]
    assert len(pairs) == 2
    return AP(base.tensor, base.offset, [pairs[0], [1, n], [1, w]])


def build_nc(n_iter=N_ITER):
    nc = bass.Bass()
    y_in = nc.declare_dram_parameter("y_slab", [ROWS, WM], f32, isOutput=False)
    m_in = nc.declare_dram_parameter("m_slab", [ROWS, W], bf16, isOutput=False)
    mi_in = nc.declare_dram_parameter("mi_slab", [ROWS, L, W], bf16, isOutput=False)
    w_in = nc.declare_dram_parameter("wmats", [128, 4, 128], bf16, isOutput=False)
    out = nc.declare_dram_parameter("xout", [L, OUT_ROWS, W], f32, isOutput=True)

    NPAIR = L // 2  # 14

    with tile.TileContext(nc) as tc:
        with (
            tc.tile_pool(name="state", bufs=1) as st,
            tc.tile_pool(name="ybps", bufs=2, space="PSUM") as ybp,
            tc.tile_pool(name="cps", bufs=2, space="PSUM") as cp,
        ):
            # ---- load inputs (small ones first; mi streams during preamble)
            y_sb = st.tile([ROWS, WM], f32)
            m_sb = st.tile([ROWS, W], bf16)
            wm = st.tile([128, 4, 128], bf16)
            mi = st.tile([ROWS, L, W], bf16)
            nc.sync.dma_start(y_sb[:], y_in[:])
            nc.sync.dma_start(m_sb[:], m_in[:])
            nc.sync.dma_start(wm[:], w_in[:])
            nc.sync.dma_start(mi[:], mi_in[:])

            W_I = wm[:, 0, :]
            W_C = [wm[:, 1 + t, :] for t in range(3)]  # col taps -1, 0, +1

            # ---- persistent state
            ybf = st.tile([ROWS, WM], bf16)
            m2_sb = st.tile([ROWS, W], bf16)
            s_sb = st.tile([ROWS, WM], f32)
            t0_sb = st.tile([ROWS, WM], bf16)
            xs = st.tile([ROWS, L, XP], bf16)
            zr = st.tile([128, L], bf16)
            wq = [st.tile([ROWS, 4, XP], bf16, name=f"wq{i}") for i in range(3)]
            vq = [st.tile([ROWS, 4, XP], bf16, name=f"vq{i}") for i in range(3)]
            up = [st.tile([ROWS, 2, W], bf16, name=f"up{i}") for i in range(4)]
            stg = [st.tile([ROWS, 2, W], f32, name=f"stg{i}") for i in range(2)]

            nc.vector.tensor_copy(ybf[:], y_sb[:])
            nc.vector.tensor_mul(out=m2_sb[:], in0=m_sb[:], in1=m_sb[:])
            nc.vector.tensor_scalar_mul(s_sb[:], y_sb[:], 2.0)
            nc.vector.memset(zr[:], 0.0)
            zp = st.tile([128, 2], bf16)
            nc.vector.memset(zp[:], 0.0)
            # zero the pad columns once; all later writes stay inside [2, 514)
            for t in (xs, *wq, *vq):
                nb = t.shape[1]
                nc.vector.tensor_copy(
                    t[:, :, 0:2], zp[:, None, :].to_broadcast((ROWS, nb, 2))
                )
                nc.vector.tensor_copy(
                    t[:, :, XP - 2 : XP], zp[:, None, :].to_broadcast((ROWS, nb, 2))
                )

            # ---- preamble: x0 = m*y[shift], u0 = (m*m)*y[shift], A(0)
            quads = [(q * 4, min(4, L - q * 4)) for q in range((L + 3) // 4)]
            for l0, n in quads:
                nc.vector.tensor_mul(
                    out=xs[:, l0 : l0 + n, 2 : 2 + W],
                    in0=m_sb[:, None, :].to_broadcast((ROWS, n, W)),
                    in1=_win3(ybf, l0, n, W),
                )

            yb_tiles = {}

            def yb_tile(k):
                if k not in yb_tiles:
                    yb_tiles[k] = ybp.tile([ROWS, WM + 5], f32, tag="yb", name=f"yb{k}")
                return yb_tiles[k]

            def emit_zero_tail(k):
                nc.tensor.matmul(
                    yb_tile(k)[:, W : W + L], W_I, zr[:],
                    start=True, stop=False, skip_group_check=True,
                )

            def emit_A_band(k, l, u_ap):
                # matmul outs must not cross the PSUM bank boundary at col 512
                yb = yb_tile(k)
                if l == 0:
                    nc.tensor.matmul(
                        yb[:, 0:W], W_I, u_ap,
                        start=True, stop=False, skip_group_check=True,
                    )
                else:
                    nc.tensor.matmul(
                        yb[:, l:W], W_I, u_ap[:, 0 : W - l],
                        start=False, stop=False, skip_group_check=True,
                    )
                    nc.tensor.matmul(
                        yb[:, W : W + l], W_I, u_ap[:, W - l : W],
                        start=False, stop=(l == L - 1), skip_group_check=True,
                    )

            # u0/A(0) from y directly: u0_l = m2 * y[l:l+W]
            emit_zero_tail(0)
            for p in range(NPAIR):
                l0 = 2 * p
                ub = up[p % 4]
                eng = nc.gpsimd if p >= NPAIR - POOL_U_PAIRS else nc.vector
                eng.tensor_mul(
                    out=ub[:],
                    in0=m2_sb[:, None, :].to_broadcast((ROWS, 2, W)),
                    in1=_win3(ybf, l0, 2, W),
                )
                emit_A_band(0, l0, ub[:, 0, :])
                emit_A_band(0, l0 + 1, ub[:, 1, :])

            # ---- iterations
            x2_tiles = {}

            def x2_tile(k, p):
                x2_tiles[(k, p)] = cp.tile([ROWS, 2, W], f32, tag="x2", name=f"x2_{k}_{p}")
                return x2_tiles[(k, p)]

            def emit_conv_mms(x2, half, rhs_tile, l, start, stop):
                # rhs_tile: [ROWS, nb, XP] (or xs) holding the band at [2, 514)
                dst = x2[:, half, :]
                nc.tensor.matmul(
                    dst, W_C[1], rhs_tile[:, l, 2 : 2 + W],
                    start=start, stop=False, skip_group_check=True,
                )
                nc.tensor.matmul(
                    dst, W_C[0], rhs_tile[:, l, 1 : 1 + W],
                    start=False, stop=False, skip_group_check=True,
                )
                nc.tensor.matmul(
                    dst, W_C[2], rhs_tile[:, l, 3 : 3 + W],
                    start=False, stop=stop, skip_group_check=True,
                )

            def emit_copy_pair(k, p, last):
                x2 = x2_tiles[(k, p)]
                l0 = 2 * p
                if last:
                    sg = stg[p % 2]
                    nc.scalar.copy(sg[:], x2[:])
                    nc.sync.dma_start(
                        out[l0, :, :], sg[HALO : HALO + OUT_ROWS, 0, :]
                    )
                    nc.sync.dma_start(
                        out[l0 + 1, :, :], sg[HALO : HALO + OUT_ROWS, 1, :]
                    )
                else:
                    nc.scalar.copy(xs[:, l0 : l0 + 2, 2 : 2 + W], x2[:])

            def emit_u_pair(k, p):
                # u_l = m * x_l  (new xs) -> feeds yb(k+1)
                l0 = 2 * p
                ub = up[p % 4]
                eng = nc.gpsimd if p >= NPAIR - POOL_U_PAIRS else nc.vector
                eng.tensor_mul(
                    out=ub[:],
                    in0=m_sb[:, None, :].to_broadcast((ROWS, 2, W)),
                    in1=xs[:, l0 : l0 + 2, 2 : 2 + W],
                )
                emit_A_band(k + 1, l0, ub[:, 0, :])
                emit_A_band(k + 1, l0 + 1, ub[:, 1, :])

            for k in range(n_iter):
                last = k == n_iter - 1
                yb = yb_tile(k)
                # boundary: conv(x) part of split bands needs no t0
                x2s = x2_tile(k, 0)
                for j in range(N_SPLIT):
                    emit_conv_mms(x2s, j, xs, j, start=True, stop=False)
                # B: t0 = s - 2*yb  (bf16 out)
                nc.vector.scalar_tensor_tensor(
                    out=t0_sb[:], in0=yb[:, 0:WM], scalar=-2.0,
                    in1=s_sb[:], op0=MUL, op1=ADD,
                )
                # v for quad 0 (bands 0..3)
                v0 = vq[0]
                nc.vector.tensor_mul(
                    out=v0[:, 0:4, 2 : 2 + W],
                    in0=mi[:, 0:4, :],
                    in1=_win3(t0_sb, 0, 4, W),
                )
                # conv(v) part of split bands
                for j in range(N_SPLIT):
                    emit_conv_mms(x2s, j, v0, j, start=False, stop=True)
                # w for bands 2,3
                w0 = wq[0]
                nc.vector.tensor_add(
                    out=w0[:, 2:4, 2 : 2 + W],
                    in0=xs[:, 2:4, 2 : 2 + W],
                    in1=v0[:, 2:4, 2 : 2 + W],
                )
                x2b = x2_tile(k, 1)
                for j in range(2):
                    emit_conv_mms(x2b, j, w0, 2 + j, start=True, stop=True)
                emit_copy_pair(k, 0, last)
                if not last:
                    emit_zero_tail(k + 1)
                # steady band loop, quads 1..6
                for q in range(1, 7):
                    l0 = 4 * q
                    vb = vq[q % 3]
                    wb = wq[q % 3]
                    nc.vector.tensor_mul(
                        out=vb[:, 0:4, 2 : 2 + W],
                        in0=mi[:, l0 : l0 + 4, :],
                        in1=_win3(t0_sb, l0, 4, W),
                    )
                    nc.vector.tensor_add(
                        out=wb[:, 0:4, 2 : 2 + W],
                        in0=xs[:, l0 : l0 + 4, 2 : 2 + W],
                        in1=vb[:, 0:4, 2 : 2 + W],
                    )
                    for pp in range(2):
                        p = 2 * q + pp
                        x2 = x2_tile(k, p)
                        for j in range(2):
                            emit_conv_mms(x2, j, wb, 2 * pp + j, start=True, stop=True)
                        emit_copy_pair(k, p - 1, last)
                        if not last and p - 2 >= 0:
                            emit_u_pair(k, p - 2)
                    if q == 2 and not last:
                        # s += y - yb, off the critical path (DVE: GPSIMD
                        # supports neither PSUM reads nor TensorScalarPtr)
                        nc.vector.scalar_tensor_tensor(
                            out=s_sb[:], in0=yb[:, 0:WM], scalar=-1.0,
                            in1=s_sb[:], op0=MUL, op1=ADD,
                        )
                        nc.vector.tensor_add(
                            out=s_sb[:], in0=s_sb[:], in1=y_sb[:]
                        )
                emit_copy_pair(k, NPAIR - 1, last)
                if not last:
                    emit_u_pair(k, NPAIR - 2)
                    emit_u_pair(k, NPAIR - 1)

    _split_excess_waits(nc, max_w=1)
    return nc


def _host_inputs(y_1hw, mask2d):
    y2 = np.asarray(y_1hw, dtype=np.float32)[0]      # [512, 539]
    m2 = np.asarray(mask2d, dtype=np.float32)        # [512, 512]
    g3 = _gauss3(SIGMA)

    # Phi_sum / mi on the full grid (host precompute; Phi depends only on m)
    Phi = np.zeros((H, WM), dtype=np.float32)
    for l in range(L):
        Phi[:, l : l + W] += m2
    Phi = np.maximum(Phi, 1.0)
    invPhi = (1.0 / Phi).astype(np.float32)

    ident = np.eye(128, dtype=np.float32)

    in_maps = []
    for c in range(NCORES):
        rk = 64 * c - HALO
        y_slab = np.zeros((ROWS, WM), dtype=np.float32)
        m_slab = np.zeros((ROWS, W), dtype=np.float32)
        mi_slab = np.zeros((ROWS, L, W), dtype=np.float32)
        lo = max(0, -rk)              # first valid slab row
        hi = min(ROWS, H - rk)        # one past last valid slab row
        y_slab[lo:hi] = y2[rk + lo : rk + hi]
        m_slab[lo:hi] = m2[rk + lo : rk + hi]
        iv = invPhi[rk + lo : rk + hi]  # [vr, WM]
        for l in range(L):
            mi_slab[lo:hi, l, :] = m_slab[lo:hi] * iv[:, l : l + W]
        # banded 3-tap row-conv matrix, zeroed outside the valid row range
        B = np.zeros((128, 128), dtype=np.float32)
        for kk in range(-1, 2):
            for i in range(128):
                ip = i + kk
                if lo <= i < hi and lo <= ip < hi:
                    B[ip, i] = g3[kk + 1]
        wmats = np.zeros((128, 4, 128), dtype=np.float32)
        wmats[:, 0, :] = ident
        for t in range(3):
            wmats[:, 1 + t, :] = g3[t] * B
        in_maps.append(
            {
                "y_slab": y_slab,
                "m_slab": m_slab.astype(ml_dtypes.bfloat16),
                "mi_slab": mi_slab.astype(ml_dtypes.bfloat16),
                "wmats": wmats.astype(ml_dtypes.bfloat16),
            }
        )
    return in_maps


_NC_CACHE = {}


def _get_nc(dx, n_iter=N_ITER):
    key = (tuple(int(v) for v in dx), n_iter)
    if key not in _NC_CACHE:
        assert all(int(d) == i for i, d in enumerate(key[0])), (
            "kernel assumes dx[l] == l"
        )
        _NC_CACHE[key] = build_nc(n_iter)
    return _NC_CACHE[key]


def kernel(y_1hw, mask2d, phi_d_deg, s_nom, n_iter=N_ITER, trace=False):
    s = np.asarray(s_nom, dtype=np.float32)
    phi = float(np.asarray(phi_d_deg))
    dx, dy = _offsets(s, phi)
    assert (dy == 0).all(), "kernel assumes dy == 0 (row shifts unsupported)"
    nc = _get_nc(dx, n_iter)
    in_maps = _host_inputs(y_1hw, mask2d)
    res = run_bass_kernel_spmd(nc, in_maps, list(range(NCORES)), trace=trace)
    x_full = np.empty((1, L, H, W), dtype=np.float32)
    for c in range(NCORES):
        x_full[0, :, 64 * c : 64 * (c + 1), :] = res.results[c]["xout"]
    kernel.last_results = res
    return x_full


# revision 7
# speedup vs baseline: 1.3878x; 1.0161x over previous
"""CASSI GAP reconstruction (DifferentiableGAPTV) on 8 Trainium2 NeuronCores.

Strategy: shard H=512 rows across 8 cores as 128-row slabs (64 output rows +
32-row halo each side).  Rows are independent except the depthwise conv
(3-tap => +-1 row/iter * 12 iters = 12-row dependency), so the halo makes the
whole 12-iteration loop collective-free; each core's central 64 rows are exact.

Numerics (validated vs the fp32 reference on CPU, rel err ~8e-3 < 2e-2):
 - 5-tap sigma=0.5 Gaussian -> renormalized 3-tap (outer taps are 2.6e-4).
 - band states, masks, and per-band elementwise ops in bf16 (DVE 2x mode);
   the measurement-plane accumulator s = y1 + y stays fp32 (updated via
   s' = 0.5*(s + t0) + y since t0 = s - 2*yb, so GPSIMD never touches PSUM).

Per-core, per iteration (bands l = 0..27, dx[l] == l):
  A:  yb = sum_l shift_l(m*x_l)  -- identity matmuls into a PSUM plane;
      u_l = m*x_l on DVE / GPSIMD (bf16); the last two band pairs read the
      conv output straight from PSUM so the boundary chain skips the copy.
  B:  t0 = s - 2*yb (DVE stt, bf16 out)
  C:  v_l = mi_l * t0[l:l+W]  (mi = m/Phi_sum, host-precomputed bf16;
      one DVE op per 4 bands via an overlapping-window AP, dx[l] == l)
      bands 0..3:  x_l' = conv(x_l) + conv(v_l) -- the conv(x_l) matmuls
        need no t0, so PE crosses the iteration boundary without idling.
      bands 4..27: w_l = x_l + v_l (DVE, 4-band ops two quads ahead of PE),
        x_l' = conv(w_l)
      conv = 2D 3x3 via 3 matmuls (row conv in the weights, col taps as
      +-1-shifted rhs windows); PSUM->SBUF copies on ACT; the A-matmuls for
      iteration k+1 interleave into iteration k's band loop.
"""
import sys

sys.path.insert(0, "/opt/trn_rl_repo")
import numpy as np
import ml_dtypes
import concourse.bass as bass
import concourse.mybir as mybir
import concourse.tile as tile
from concourse.bass_utils import run_bass_kernel_spmd
from bass_rust import AP

H, W, L = 512, 512, 28
N_ITER = 12
SIGMA = 0.5
PI = 3.141592653589793
NCORES = 8
ROWS = 128          # slab rows per core
OUT_ROWS = 64       # exact output rows per core
HALO = 32           # (ROWS - OUT_ROWS) / 2
WM = W + L - 1      # measurement-plane width (539)
XP = W + 4          # padded band pitch (2 zero cols each side)

f32 = mybir.dt.float32
bf16 = mybir.dt.bfloat16
MUL = mybir.AluOpType.mult
ADD = mybir.AluOpType.add

NPAIR = L // 2           # 14 band pairs
POOL_U_PAIRS = (7, 8, 9, 10, 11)   # u pairs computed on GPSIMD
PSUM_U_PAIRS = (12, 13)  # u pairs reading conv PSUM directly (DVE)


def _offsets(s, phi_deg):
    phi = phi_deg * PI / 180.0
    dx = s * np.cos(phi)
    dy = s * np.sin(phi)
    dx = dx - dx.min()
    dy = dy - dy.min()
    return np.rint(dx).astype(np.int32), np.rint(dy).astype(np.int32)


def _gauss3(sigma):
    ksize = max(3, int(6 * sigma + 1) | 1)
    ax = np.arange(ksize, dtype=np.float32) - ksize // 2
    g1 = np.exp(-0.5 * (ax / sigma) ** 2)
    g1 = g1 / g1.sum()
    c = ksize // 2
    g3 = g1[c - 1 : c + 2].astype(np.float64)
    g3 = (g3 / g3.sum()).astype(np.float32)
    return g3  # [3]


def _split_excess_waits(nc, max_w=1):
    """walrus in this toolchain accepts at most one sync wait per instruction;
    hoist excess waits onto preceding same-engine NoOp carriers."""
    ctr = 0
    for f in nc.m.functions:
        for bb in f.blocks:
            il = bb.instructions
            i = 0
            while i < len(il):
                inst = il[i]
                si = inst.sync_info
                w = list(si.on_wait) if (si and si.on_wait) else []
                if len(w) > max_w:
                    si.on_wait = w[-max_w:]
                    extra = w[:-max_w]
                    pos = i
                    for j in range(0, len(extra), max_w):
                        ctr += 1
                        nop = mybir.InstNoOp(
                            name=f"I-waitsplit-{ctr}", ins=[], outs=[]
                        )
                        nop.engine = inst.engine
                        nop.sync_info = mybir.SyncInfo(
                            on_wait=extra[j : j + max_w], on_update=[]
                        )
                        il.insert(pos, nop)
                        pos += 1
                        i += 1
                i += 1


def _win3(tile2d, l0, n, w):
    """[128, n, w] overlapping-window view of a [128, >=l0+n-1+w] tile:
    out[:, j, c] = tile2d[:, l0 + j + c]  (band axis stride = 1 element)."""
    base = tile2d[:, l0 : l0 + w]
    pairs = [list(p) for p in base.ap]
    assert len(pairs) == 2
    return AP(base.tensor, base.offset, [pairs[0], [1, n], [1, w]])


def build_nc(n_iter=N_ITER):
    nc = bass.Bass()
    y_in = nc.declare_dram_parameter("y_slab", [ROWS, WM], f32, isOutput=False)
    m_in = nc.declare_dram_parameter("m_slab", [ROWS, W], bf16, isOutput=False)
    mi_in = nc.declare_dram_parameter("mi_slab", [ROWS, L, W], bf16, isOutput=False)
    w_in = nc.declare_dram_parameter("wmats", [128, 4, 128], bf16, isOutput=False)
    out = nc.declare_dram_parameter("xout", [L, OUT_ROWS, W], f32, isOutput=True)

    with tile.TileContext(nc) as tc:
        with (
            tc.tile_pool(name="state", bufs=1) as st,
            tc.tile_pool(name="ybps", bufs=2, space="PSUM") as ybp,
            tc.tile_pool(name="cps", bufs=4, space="PSUM") as cp,
        ):
            # ---- load inputs (small ones first; mi streams during preamble)
            y_sb = st.tile([ROWS, WM], f32)
            m_sb = st.tile([ROWS, W], bf16)
            wm = st.tile([128, 4, 128], bf16)
            mi = st.tile([ROWS, L, W], bf16)
            nc.sync.dma_start(y_sb[:], y_in[:])
            nc.sync.dma_start(m_sb[:], m_in[:])
            nc.sync.dma_start(wm[:], w_in[:])
            nc.sync.dma_start(mi[:], mi_in[:])

            W_I = wm[:, 0, :]
            W_C = [wm[:, 1 + t, :] for t in range(3)]  # col taps -1, 0, +1

            # ---- persistent state
            ybf = st.tile([ROWS, WM], bf16)
            m2_sb = st.tile([ROWS, W], bf16)
            s_sb = st.tile([ROWS, WM], f32)
            stmp = st.tile([ROWS, WM], f32)
            half = st.tile([ROWS, 1], f32)
            t0_sb = st.tile([ROWS, WM], bf16)
            xs = st.tile([ROWS, L, XP], bf16)
            zr = st.tile([128, L], bf16)
            wq = [st.tile([ROWS, 4, XP], bf16, name=f"wq{i}") for i in range(3)]
            vq = [st.tile([ROWS, 4, XP], bf16, name=f"vq{i}") for i in range(3)]
            up = [st.tile([ROWS, 2, W], bf16, name=f"up{i}") for i in range(4)]
            stg = [st.tile([ROWS, W], f32, name=f"stg{i}") for i in range(3)]

            nc.vector.tensor_copy(ybf[:], y_sb[:])
            nc.vector.tensor_mul(out=m2_sb[:], in0=m_sb[:], in1=m_sb[:])
            nc.vector.tensor_scalar_mul(s_sb[:], y_sb[:], 2.0)
            nc.vector.memset(half[:], 0.5)
            nc.vector.memset(zr[:], 0.0)
            zp = st.tile([128, 2], bf16)
            nc.vector.memset(zp[:], 0.0)
            # zero the pad columns once; all later writes stay inside [2, 514)
            for t in (xs, *wq, *vq):
                nb = t.shape[1]
                nc.vector.tensor_copy(
                    t[:, :, 0:2], zp[:, None, :].to_broadcast((ROWS, nb, 2))
                )
                nc.vector.tensor_copy(
                    t[:, :, XP - 2 : XP], zp[:, None, :].to_broadcast((ROWS, nb, 2))
                )

            yb_tiles = {}

            def yb_tile(k):
                if k not in yb_tiles:
                    yb_tiles[k] = ybp.tile(
                        [ROWS, WM + 5], f32, tag="yb", name=f"yb{k}"
                    )
                return yb_tiles[k]

            def emit_zero_tail(k):
                nc.tensor.matmul(
                    yb_tile(k)[:, W : W + L], W_I, zr[:],
                    start=True, stop=False, skip_group_check=True,
                )

            def emit_A_band(k, l, u_ap):
                # matmul outs must not cross the PSUM bank boundary at col 512
                yb = yb_tile(k)
                if l == 0:
                    nc.tensor.matmul(
                        yb[:, 0:W], W_I, u_ap,
                        start=True, stop=False, skip_group_check=True,
                    )
                else:
                    nc.tensor.matmul(
                        yb[:, l:W], W_I, u_ap[:, 0 : W - l],
                        start=False, stop=False, skip_group_check=True,
                    )
                    nc.tensor.matmul(
                        yb[:, W : W + l], W_I, u_ap[:, W - l : W],
                        start=False, stop=(l == L - 1), skip_group_check=True,
                    )

            def u_engine(p):
                return nc.gpsimd if p in POOL_U_PAIRS else nc.vector

            # ---- preamble: x0 = m*y[shift], u0 = (m*m)*y[shift], A(0)
            emit_zero_tail(0)
            for q in range(7):
                l0 = 4 * q
                nc.vector.tensor_mul(
                    out=xs[:, l0 : l0 + 4, 2 : 2 + W],
                    in0=m_sb[:, None, :].to_broadcast((ROWS, 4, W)),
                    in1=_win3(ybf, l0, 4, W),
                )
                for p in (2 * q, 2 * q + 1):
                    ub = up[p % 4]
                    u_engine(p).tensor_mul(
                        out=ub[:],
                        in0=m2_sb[:, None, :].to_broadcast((ROWS, 2, W)),
                        in1=_win3(ybf, 2 * p, 2, W),
                    )
                    emit_A_band(0, 2 * p, ub[:, 0, :])
                    emit_A_band(0, 2 * p + 1, ub[:, 1, :])

            # ---- iterations
            x2_tiles = {}

            def x2_tile(k, b):
                x2_tiles[(k, b)] = cp.tile([ROWS, W], f32, tag="x2", name=f"x2_{k}_{b}")
                return x2_tiles[(k, b)]

            def emit_conv_mms(x2, rhs_tile, idx, start, stop):
                # rhs_tile: [ROWS, nb, XP] holding the band at cols [2, 514)
                nc.tensor.matmul(
                    x2[:], W_C[1], rhs_tile[:, idx, 2 : 2 + W],
                    start=start, stop=False, skip_group_check=True,
                )
                nc.tensor.matmul(
                    x2[:], W_C[0], rhs_tile[:, idx, 1 : 1 + W],
                    start=False, stop=False, skip_group_check=True,
                )
                nc.tensor.matmul(
                    x2[:], W_C[2], rhs_tile[:, idx, 3 : 3 + W],
                    start=False, stop=stop, skip_group_check=True,
                )

            def emit_copy_band(k, b, last):
                x2 = x2_tiles[(k, b)]
                if last:
                    sg = stg[b % 3]
                    nc.scalar.copy(sg[:], x2[:])
                    nc.sync.dma_start(out[b, :, :], sg[HALO : HALO + OUT_ROWS, :])
                else:
                    nc.scalar.copy(xs[:, b, 2 : 2 + W], x2[:])

            def emit_u_pair(k, p):
                # u_l = m * x_l (new xs; PSUM-direct for the tail pairs)
                # -> feeds yb(k+1); A-matmuls for the tail pairs are deferred
                # to the top of iteration k+1 where they fill the PE boundary.
                l0 = 2 * p
                ub = up[p % 4]
                if p in PSUM_U_PAIRS:
                    for j in range(2):
                        nc.vector.tensor_mul(
                            out=ub[:, j, :], in0=m_sb[:],
                            in1=x2_tiles[(k, l0 + j)][:],
                        )
                else:
                    u_engine(p).tensor_mul(
                        out=ub[:],
                        in0=m_sb[:, None, :].to_broadcast((ROWS, 2, W)),
                        in1=xs[:, l0 : l0 + 2, 2 : 2 + W],
                    )
                if p not in PSUM_U_PAIRS:
                    emit_A_band(k + 1, l0, ub[:, 0, :])
                    emit_A_band(k + 1, l0 + 1, ub[:, 1, :])

            def emit_v(k, q):
                vb = vq[q % 3]
                nc.vector.tensor_mul(
                    out=vb[:, 0:4, 2 : 2 + W],
                    in0=mi[:, 4 * q : 4 * q + 4, :],
                    in1=_win3(t0_sb, 4 * q, 4, W),
                )

            def emit_w(k, q):
                wb = wq[q % 3]
                nc.vector.tensor_add(
                    out=wb[:, 0:4, 2 : 2 + W],
                    in0=xs[:, 4 * q : 4 * q + 4, 2 : 2 + W],
                    in1=vq[q % 3][:, 0:4, 2 : 2 + W],
                )

            for k in range(n_iter):
                first = k == 0
                last = k == n_iter - 1
                yb = yb_tile(k)
                # boundary: conv(x) part of split bands 0..3 needs no t0
                for b in range(4):
                    emit_conv_mms(x2_tile(k, b), xs, b, start=True, stop=False)
                if not first:
                    # deferred tail A-matmuls of yb(k) (u from PSUM, iter k-1)
                    for p in PSUM_U_PAIRS:
                        emit_A_band(k, 2 * p, up[p % 4][:, 0, :])
                        emit_A_band(k, 2 * p + 1, up[p % 4][:, 1, :])
                # B: t0 = s - 2*yb  (bf16 out)
                nc.vector.scalar_tensor_tensor(
                    out=t0_sb[:], in0=yb[:, 0:WM], scalar=-2.0,
                    in1=s_sb[:], op0=MUL, op1=ADD,
                )
                emit_v(k, 0)
                for b in range(4):
                    emit_conv_mms(x2_tiles[(k, b)], vq[0], b, start=False, stop=True)
                for b in range(4):
                    emit_copy_band(k, b, last)
                # prime the DVE lookahead: v/w two quads ahead of PE
                emit_v(k, 1)
                emit_w(k, 1)
                emit_v(k, 2)
                emit_w(k, 2)
                if not last:
                    emit_zero_tail(k + 1)
                    emit_u_pair(k, 0)
                    emit_u_pair(k, 1)
                for q in range(1, 7):
                    for b in range(4 * q, 4 * q + 4):
                        emit_conv_mms(
                            x2_tile(k, b), wq[q % 3], b - 4 * q,
                            start=True, stop=True,
                        )
                        emit_copy_band(k, b, last)
                    if q + 2 <= 6:
                        emit_v(k, q + 2)
                        emit_w(k, q + 2)
                    if not last and q < 6:
                        emit_u_pair(k, 2 * q)
                        emit_u_pair(k, 2 * q + 1)
                    if q == 2 and not last:
                        # s' = 0.5*(s + t0) + y  == s + y - yb, on GPSIMD
                        # (reads only SBUF), off the critical path
                        nc.gpsimd.tensor_add(
                            out=stmp[:], in0=s_sb[:], in1=t0_sb[:]
                        )
                        nc.gpsimd.tensor_mul(
                            out=s_sb[:], in0=stmp[:],
                            in1=half[:, 0:1].to_broadcast((ROWS, WM)),
                        )
                        nc.gpsimd.tensor_add(
                            out=s_sb[:], in0=s_sb[:], in1=y_sb[:]
                        )
                if not last:
                    # tail pairs: u reads conv PSUM directly; A deferred
                    emit_u_pair(k, NPAIR - 2)
                    emit_u_pair(k, NPAIR - 1)

    _split_excess_waits(nc, max_w=1)
    return nc


def _host_inputs(y_1hw, mask2d):
    y2 = np.asarray(y_1hw, dtype=np.float32)[0]      # [512, 539]
    m2 = np.asarray(mask2d, dtype=np.float32)        # [512, 512]
    g3 = _gauss3(SIGMA)

    # Phi_sum / mi on the full grid (host precompute; Phi depends only on m)
    Phi = np.zeros((H, WM), dtype=np.float32)
    for l in range(L):
        Phi[:, l : l + W] += m2
    Phi = np.maximum(Phi, 1.0)
    invPhi = (1.0 / Phi).astype(np.float32)

    ident = np.eye(128, dtype=np.float32)

    in_maps = []
    for c in range(NCORES):
        rk = 64 * c - HALO
        y_slab = np.zeros((ROWS, WM), dtype=np.float32)
        m_slab = np.zeros((ROWS, W), dtype=np.float32)
        mi_slab = np.zeros((ROWS, L, W), dtype=np.float32)
        lo = max(0, -rk)              # first valid slab row
        hi = min(ROWS, H - rk)        # one past last valid slab row
        y_slab[lo:hi] = y2[rk + lo : rk + hi]
        m_slab[lo:hi] = m2[rk + lo : rk + hi]
        iv = invPhi[rk + lo : rk + hi]  # [vr, WM]
        for l in range(L):
            mi_slab[lo:hi, l, :] = m_slab[lo:hi] * iv[:, l : l + W]
        # banded 3-tap row-conv matrix, zeroed outside the valid row range
        B = np.zeros((128, 128), dtype=np.float32)
        for kk in range(-1, 2):
            for i in range(128):
                ip = i + kk
                if lo <= i < hi and lo <= ip < hi:
                    B[ip, i] = g3[kk + 1]
        wmats = np.zeros((128, 4, 128), dtype=np.float32)
        wmats[:, 0, :] = ident
        for t in range(3):
            wmats[:, 1 + t, :] = g3[t] * B
        in_maps.append(
            {
                "y_slab": y_slab,
                "m_slab": m_slab.astype(ml_dtypes.bfloat16),
                "mi_slab": mi_slab.astype(ml_dtypes.bfloat16),
                "wmats": wmats.astype(ml_dtypes.bfloat16),
            }
        )
    return in_maps


_NC_CACHE = {}


def _get_nc(dx, n_iter=N_ITER):
    key = (tuple(int(v) for v in dx), n_iter)
    if key not in _NC_CACHE:
        assert all(int(d) == i for i, d in enumerate(key[0])), (
            "kernel assumes dx[l] == l"
        )
        _NC_CACHE[key] = build_nc(n_iter)
    return _NC_CACHE[key]


def kernel(y_1hw, mask2d, phi_d_deg, s_nom, n_iter=N_ITER, trace=False):
    s = np.asarray(s_nom, dtype=np.float32)
    phi = float(np.asarray(phi_d_deg))
    dx, dy = _offsets(s, phi)
    assert (dy == 0).all(), "kernel assumes dy == 0 (row shifts unsupported)"
    nc = _get_nc(dx, n_iter)
    in_maps = _host_inputs(y_1hw, mask2d)
    res = run_bass_kernel_spmd(nc, in_maps, list(range(NCORES)), trace=trace)
    x_full = np.empty((1, L, H, W), dtype=np.float32)
    for c in range(NCORES):
        x_full[0, :, 64 * c : 64 * (c + 1), :] = res.results[c]["xout"]
    kernel.last_results = res
    return x_full


# revision 8
# speedup vs baseline: 1.4924x; 1.0754x over previous
"""CASSI GAP reconstruction (DifferentiableGAPTV) on 8 Trainium2 NeuronCores.

Strategy: shard H=512 rows across 8 cores as 128-row slabs (64 output rows +
32-row halo each side).  Rows are independent except the depthwise conv
(3-tap => +-1 row/iter * 12 iters = 12-row dependency), so the halo makes the
whole 12-iteration loop collective-free; each core's central 64 rows are exact.

Numerics (validated vs the fp32 reference on CPU, rel err ~8e-3 < 2e-2):
 - 5-tap sigma=0.5 Gaussian -> renormalized 3-tap (outer taps are 2.6e-4).
 - band states, masks, and per-band elementwise ops in bf16 (DVE 2x mode);
   the measurement-plane accumulator s = y1 + y stays fp32 (updated via
   s' = 0.5*(s + t0) + y since t0 = s - 2*yb, so GPSIMD never touches PSUM).

Per-core, per iteration (bands l = 0..27, dx[l] == l):
  A:  yb = sum_l shift_l(m*x_l)  -- identity matmuls into a PSUM plane;
      u_l = m*x_l on DVE / GPSIMD (bf16); the last two band pairs read the
      conv output straight from PSUM so the boundary chain skips the copy.
  B:  t0 = s - 2*yb (DVE stt, bf16 out)
  C:  v_l = mi_l * t0[l:l+W]  (mi = m/Phi_sum, host-precomputed bf16;
      one DVE op per 4 bands via an overlapping-window AP, dx[l] == l)
      bands 0..3:  x_l' = conv(x_l) + conv(v_l) -- the conv(x_l) matmuls
        need no t0, so PE crosses the iteration boundary without idling.
      bands 4..27: w_l = x_l + v_l (DVE, 4-band ops two quads ahead of PE),
        x_l' = conv(w_l)
      conv = 2D 3x3 via 3 matmuls (row conv in the weights, col taps as
      +-1-shifted rhs windows); PSUM->SBUF copies on ACT; the A-matmuls for
      iteration k+1 interleave into iteration k's band loop.
"""
import sys

sys.path.insert(0, "/opt/trn_rl_repo")
import numpy as np
import ml_dtypes
import concourse.bass as bass
import concourse.mybir as mybir
import concourse.tile as tile
from concourse.bass_utils import run_bass_kernel_spmd
from bass_rust import AP

H, W, L = 512, 512, 28
N_ITER = 12
SIGMA = 0.5
PI = 3.141592653589793
NCORES = 8
ROWS = 128          # slab rows per core
OUT_ROWS = 64       # exact output rows per core
HALO = 32           # (ROWS - OUT_ROWS) / 2
WM = W + L - 1      # measurement-plane width (539)
XP = W + 4          # padded band pitch (2 zero cols each side)

f32 = mybir.dt.float32
bf16 = mybir.dt.bfloat16
MUL = mybir.AluOpType.mult
ADD = mybir.AluOpType.add

NPAIR = L // 2           # 14 band pairs
POOL_U_PAIRS = (7, 8, 9, 10, 11)   # u pairs computed on GPSIMD
PSUM_U_PAIRS = (12, 13)  # u pairs reading conv PSUM directly (DVE)


def _offsets(s, phi_deg):
    phi = phi_deg * PI / 180.0
    dx = s * np.cos(phi)
    dy = s * np.sin(phi)
    dx = dx - dx.min()
    dy = dy - dy.min()
    return np.rint(dx).astype(np.int32), np.rint(dy).astype(np.int32)


def _gauss3(sigma):
    ksize = max(3, int(6 * sigma + 1) | 1)
    ax = np.arange(ksize, dtype=np.float32) - ksize // 2
    g1 = np.exp(-0.5 * (ax / sigma) ** 2)
    g1 = g1 / g1.sum()
    c = ksize // 2
    g3 = g1[c - 1 : c + 2].astype(np.float64)
    g3 = (g3 / g3.sum()).astype(np.float32)
    return g3  # [3]


def _split_excess_waits(nc, max_w=1):
    """walrus in this toolchain accepts at most one sync wait per instruction;
    hoist excess waits onto preceding same-engine NoOp carriers."""
    ctr = 0
    for f in nc.m.functions:
        for bb in f.blocks:
            il = bb.instructions
            i = 0
            while i < len(il):
                inst = il[i]
                si = inst.sync_info
                w = list(si.on_wait) if (si and si.on_wait) else []
                if len(w) > max_w:
                    si.on_wait = w[-max_w:]
                    extra = w[:-max_w]
                    pos = i
                    for j in range(0, len(extra), max_w):
                        ctr += 1
                        nop = mybir.InstNoOp(
                            name=f"I-waitsplit-{ctr}", ins=[], outs=[]
                        )
                        nop.engine = inst.engine
                        nop.sync_info = mybir.SyncInfo(
                            on_wait=extra[j : j + max_w], on_update=[]
                        )
                        il.insert(pos, nop)
                        pos += 1
                        i += 1
                i += 1


def _win3(tile2d, l0, n, w):
    """[128, n, w] overlapping-window view of a [128, >=l0+n-1+w] tile:
    out[:, j, c] = tile2d[:, l0 + j + c]  (band axis stride = 1 element)."""
    base = tile2d[:, l0 : l0 + w]
    pairs = [list(p) for p in base.ap]
    assert len(pairs) == 2
    return AP(base.tensor, base.offset, [pairs[0], [1, n], [1, w]])


def build_nc(n_iter=N_ITER):
    nc = bass.Bass()
    y_in = nc.declare_dram_parameter("y_slab", [ROWS, WM], f32, isOutput=False)
    m_in = nc.declare_dram_parameter("m_slab", [ROWS, W], bf16, isOutput=False)
    mi_in = nc.declare_dram_parameter("mi_slab", [ROWS, L, W], bf16, isOutput=False)
    w_in = nc.declare_dram_parameter("wmats", [128, 4, 128], bf16, isOutput=False)
    out = nc.declare_dram_parameter("xout", [L, OUT_ROWS, W], f32, isOutput=True)

    with tile.TileContext(nc) as tc:
        with (
            tc.tile_pool(name="state", bufs=1) as st,
            tc.tile_pool(name="ybps", bufs=2, space="PSUM") as ybp,
            tc.tile_pool(name="cps", bufs=4, space="PSUM") as cp,
        ):
            # ---- load inputs (small ones first; mi streams during preamble)
            y_sb = st.tile([ROWS, WM], f32)
            m_sb = st.tile([ROWS, W], bf16)
            wm = st.tile([128, 4, 128], bf16)
            mi = st.tile([ROWS, L, W], bf16)
            nc.sync.dma_start(y_sb[:], y_in[:])
            nc.sync.dma_start(m_sb[:], m_in[:])
            nc.sync.dma_start(wm[:], w_in[:])
            nc.sync.dma_start(mi[:], mi_in[:])

            W_I = wm[:, 0, :]
            W_C = [wm[:, 1 + t, :] for t in range(3)]  # col taps -1, 0, +1

            # ---- persistent state
            ybf = st.tile([ROWS, WM], bf16)
            m2_sb = st.tile([ROWS, W], bf16)
            s_sb = st.tile([ROWS, WM], f32)
            stmp = st.tile([ROWS, WM], f32)
            half = st.tile([ROWS, 1], f32)
            t0_sb = st.tile([ROWS, WM], bf16)
            xs = st.tile([ROWS, L, XP], bf16)
            zr = st.tile([128, L], bf16)
            wq = [st.tile([ROWS, 4, XP], bf16, name=f"wq{i}") for i in range(3)]
            vq = [st.tile([ROWS, 4, XP], bf16, name=f"vq{i}") for i in range(3)]
            up = [st.tile([ROWS, 2, W], bf16, name=f"up{i}") for i in range(4)]
            stg = [st.tile([ROWS, W], f32, name=f"stg{i}") for i in range(3)]

            nc.vector.tensor_copy(ybf[:], y_sb[:])
            nc.vector.tensor_mul(out=m2_sb[:], in0=m_sb[:], in1=m_sb[:])
            nc.vector.tensor_scalar_mul(s_sb[:], y_sb[:], 2.0)
            nc.vector.memset(half[:], 0.5)
            nc.vector.memset(zr[:], 0.0)
            zp = st.tile([128, 2], bf16)
            nc.vector.memset(zp[:], 0.0)
            # zero the pad columns once; all later writes stay inside [2, 514)
            for t in (xs, *wq, *vq):
                nb = t.shape[1]
                nc.vector.tensor_copy(
                    t[:, :, 0:2], zp[:, None, :].to_broadcast((ROWS, nb, 2))
                )
                nc.vector.tensor_copy(
                    t[:, :, XP - 2 : XP], zp[:, None, :].to_broadcast((ROWS, nb, 2))
                )

            yb_tiles = {}

            def yb_tile(k):
                if k not in yb_tiles:
                    yb_tiles[k] = ybp.tile(
                        [ROWS, WM + 5], f32, tag="yb", name=f"yb{k}"
                    )
                return yb_tiles[k]

            def emit_zero_tail(k):
                nc.tensor.matmul(
                    yb_tile(k)[:, W : W + L], W_I, zr[:],
                    start=True, stop=False, skip_group_check=True,
                )

            def emit_A_band(k, l, u_ap):
                # matmul outs must not cross the PSUM bank boundary at col 512
                yb = yb_tile(k)
                if l == 0:
                    nc.tensor.matmul(
                        yb[:, 0:W], W_I, u_ap,
                        start=True, stop=False, skip_group_check=True,
                    )
                else:
                    nc.tensor.matmul(
                        yb[:, l:W], W_I, u_ap[:, 0 : W - l],
                        start=False, stop=False, skip_group_check=True,
                    )
                    nc.tensor.matmul(
                        yb[:, W : W + l], W_I, u_ap[:, W - l : W],
                        start=False, stop=(l == L - 1), skip_group_check=True,
                    )

            def u_engine(p):
                return nc.gpsimd if p in POOL_U_PAIRS else nc.vector

            # ---- preamble: x0 = m*y[shift], u0 = (m*m)*y[shift], A(0)
            emit_zero_tail(0)
            for q in range(7):
                l0 = 4 * q
                nc.vector.tensor_mul(
                    out=xs[:, l0 : l0 + 4, 2 : 2 + W],
                    in0=m_sb[:, None, :].to_broadcast((ROWS, 4, W)),
                    in1=_win3(ybf, l0, 4, W),
                )
                for p in (2 * q, 2 * q + 1):
                    ub = up[p % 4]
                    u_engine(p).tensor_mul(
                        out=ub[:],
                        in0=m2_sb[:, None, :].to_broadcast((ROWS, 2, W)),
                        in1=_win3(ybf, 2 * p, 2, W),
                    )
                    emit_A_band(0, 2 * p, ub[:, 0, :])
                    emit_A_band(0, 2 * p + 1, ub[:, 1, :])

            # ---- iterations
            x2_tiles = {}

            def x2_tile(k, b):
                x2_tiles[(k, b)] = cp.tile([ROWS, W], f32, tag="x2", name=f"x2_{k}_{b}")
                return x2_tiles[(k, b)]

            def emit_conv_mms(x2, rhs_tile, idx, start, stop):
                # rhs_tile: [ROWS, nb, XP] holding the band at cols [2, 514)
                nc.tensor.matmul(
                    x2[:], W_C[1], rhs_tile[:, idx, 2 : 2 + W],
                    start=start, stop=False, skip_group_check=True,
                )
                nc.tensor.matmul(
                    x2[:], W_C[0], rhs_tile[:, idx, 1 : 1 + W],
                    start=False, stop=False, skip_group_check=True,
                )
                nc.tensor.matmul(
                    x2[:], W_C[2], rhs_tile[:, idx, 3 : 3 + W],
                    start=False, stop=stop, skip_group_check=True,
                )

            def emit_copy_band(k, b, last):
                x2 = x2_tiles[(k, b)]
                if last:
                    sg = stg[b % 3]
                    nc.scalar.copy(sg[:], x2[:])
                    nc.sync.dma_start(out[b, :, :], sg[HALO : HALO + OUT_ROWS, :])
                else:
                    nc.scalar.copy(xs[:, b, 2 : 2 + W], x2[:])

            def emit_u_pair(k, p):
                # u_l = m * x_l (new xs; PSUM-direct for the tail pairs)
                # -> feeds yb(k+1); A-matmuls for the tail pairs are deferred
                # to the top of iteration k+1 where they fill the PE boundary.
                l0 = 2 * p
                ub = up[p % 4]
                if p in PSUM_U_PAIRS:
                    for j in range(2):
                        nc.vector.tensor_mul(
                            out=ub[:, j, :], in0=m_sb[:],
                            in1=x2_tiles[(k, l0 + j)][:],
                        )
                else:
                    u_engine(p).tensor_mul(
                        out=ub[:],
                        in0=m_sb[:, None, :].to_broadcast((ROWS, 2, W)),
                        in1=xs[:, l0 : l0 + 2, 2 : 2 + W],
                    )
                if p not in PSUM_U_PAIRS:
                    emit_A_band(k + 1, l0, ub[:, 0, :])
                    emit_A_band(k + 1, l0 + 1, ub[:, 1, :])

            def emit_v(k, q):
                vb = vq[q % 3]
                nc.vector.tensor_mul(
                    out=vb[:, 0:4, 2 : 2 + W],
                    in0=mi[:, 4 * q : 4 * q + 4, :],
                    in1=_win3(t0_sb, 4 * q, 4, W),
                )

            def emit_w(k, q):
                wb = wq[q % 3]
                nc.vector.tensor_add(
                    out=wb[:, 0:4, 2 : 2 + W],
                    in0=xs[:, 4 * q : 4 * q + 4, 2 : 2 + W],
                    in1=vq[q % 3][:, 0:4, 2 : 2 + W],
                )

            for k in range(n_iter):
                first = k == 0
                last = k == n_iter - 1
                yb = yb_tile(k)
                # boundary: conv(x) part of split bands 0..3 needs no t0
                for b in range(4):
                    emit_conv_mms(x2_tile(k, b), xs, b, start=True, stop=False)
                if not first:
                    # deferred tail A-matmuls of yb(k) (u from PSUM, iter k-1)
                    for p in PSUM_U_PAIRS:
                        emit_A_band(k, 2 * p, up[p % 4][:, 0, :])
                        emit_A_band(k, 2 * p + 1, up[p % 4][:, 1, :])
                # B: t0 = s - 2*yb  (bf16 out)
                nc.vector.scalar_tensor_tensor(
                    out=t0_sb[:], in0=yb[:, 0:WM], scalar=-2.0,
                    in1=s_sb[:], op0=MUL, op1=ADD,
                )
                emit_v(k, 0)
                for b in range(4):
                    emit_conv_mms(x2_tiles[(k, b)], vq[0], b, start=False, stop=True)
                for b in range(4):
                    emit_copy_band(k, b, last)
                # prime the DVE lookahead: v/w two quads ahead of PE
                emit_v(k, 1)
                emit_w(k, 1)
                emit_v(k, 2)
                emit_w(k, 2)
                if not last:
                    emit_zero_tail(k + 1)
                for q in range(1, 7):
                    for b in range(4 * q, 4 * q + 4):
                        emit_conv_mms(
                            x2_tile(k, b), wq[q % 3], b - 4 * q,
                            start=True, stop=True,
                        )
                        emit_copy_band(k, b, last)
                    if q + 2 <= 6:
                        emit_v(k, q + 2)
                        emit_w(k, q + 2)
                    if not last:
                        # u/A for quad q-1's pairs: their copies finished a
                        # full quad ago, so they never block the v/w stream
                        # on the in-order DVE queue
                        emit_u_pair(k, 2 * (q - 1))
                        emit_u_pair(k, 2 * (q - 1) + 1)
                    if q == 2 and not last:
                        # s' = 0.5*(s + t0) + y  == s + y - yb, on GPSIMD
                        # (reads only SBUF), off the critical path
                        nc.gpsimd.tensor_add(
                            out=stmp[:], in0=s_sb[:], in1=t0_sb[:]
                        )
                        nc.gpsimd.tensor_mul(
                            out=s_sb[:], in0=stmp[:],
                            in1=half[:, 0:1].to_broadcast((ROWS, WM)),
                        )
                        nc.gpsimd.tensor_add(
                            out=s_sb[:], in0=s_sb[:], in1=y_sb[:]
                        )
                if not last:
                    # tail pairs: u reads conv PSUM directly; A deferred
                    emit_u_pair(k, NPAIR - 2)
                    emit_u_pair(k, NPAIR - 1)

    _split_excess_waits(nc, max_w=1)
    return nc


def _host_inputs(y_1hw, mask2d):
    y2 = np.asarray(y_1hw, dtype=np.float32)[0]      # [512, 539]
    m2 = np.asarray(mask2d, dtype=np.float32)        # [512, 512]
    g3 = _gauss3(SIGMA)

    # Phi_sum / mi on the full grid (host precompute; Phi depends only on m)
    Phi = np.zeros((H, WM), dtype=np.float32)
    for l in range(L):
        Phi[:, l : l + W] += m2
    Phi = np.maximum(Phi, 1.0)
    invPhi = (1.0 / Phi).astype(np.float32)

    ident = np.eye(128, dtype=np.float32)

    in_maps = []
    for c in range(NCORES):
        rk = 64 * c - HALO
        y_slab = np.zeros((ROWS, WM), dtype=np.float32)
        m_slab = np.zeros((ROWS, W), dtype=np.float32)
        mi_slab = np.zeros((ROWS, L, W), dtype=np.float32)
        lo = max(0, -rk)              # first valid slab row
        hi = min(ROWS, H - rk)        # one past last valid slab row
        y_slab[lo:hi] = y2[rk + lo : rk + hi]
        m_slab[lo:hi] = m2[rk + lo : rk + hi]
        iv = invPhi[rk + lo : rk + hi]  # [vr, WM]
        for l in range(L):
            mi_slab[lo:hi, l, :] = m_slab[lo:hi] * iv[:, l : l + W]
        # banded 3-tap row-conv matrix, zeroed outside the valid row range
        B = np.zeros((128, 128), dtype=np.float32)
        for kk in range(-1, 2):
            for i in range(128):
                ip = i + kk
                if lo <= i < hi and lo <= ip < hi:
                    B[ip, i] = g3[kk + 1]
        wmats = np.zeros((128, 4, 128), dtype=np.float32)
        wmats[:, 0, :] = ident
        for t in range(3):
            wmats[:, 1 + t, :] = g3[t] * B
        in_maps.append(
            {
                "y_slab": y_slab,
                "m_slab": m_slab.astype(ml_dtypes.bfloat16),
                "mi_slab": mi_slab.astype(ml_dtypes.bfloat16),
                "wmats": wmats.astype(ml_dtypes.bfloat16),
            }
        )
    return in_maps


_NC_CACHE = {}


def _get_nc(dx, n_iter=N_ITER):
    key = (tuple(int(v) for v in dx), n_iter)
    if key not in _NC_CACHE:
        assert all(int(d) == i for i, d in enumerate(key[0])), (
            "kernel assumes dx[l] == l"
        )
        _NC_CACHE[key] = build_nc(n_iter)
    return _NC_CACHE[key]


def kernel(y_1hw, mask2d, phi_d_deg, s_nom, n_iter=N_ITER, trace=False):
    s = np.asarray(s_nom, dtype=np.float32)
    phi = float(np.asarray(phi_d_deg))
    dx, dy = _offsets(s, phi)
    assert (dy == 0).all(), "kernel assumes dy == 0 (row shifts unsupported)"
    nc = _get_nc(dx, n_iter)
    in_maps = _host_inputs(y_1hw, mask2d)
    res = run_bass_kernel_spmd(nc, in_maps, list(range(NCORES)), trace=trace)
    x_full = np.empty((1, L, H, W), dtype=np.float32)
    for c in range(NCORES):
        x_full[0, :, 64 * c : 64 * (c + 1), :] = res.results[c]["xout"]
    kernel.last_results = res
    return x_full
